# revision 33
# baseline (speedup 1.0000x reference)
"""Trainium2 Bass kernel for nn_DeepManualLSTM (3-layer LSTM, B=1024, T=48, IN=64, H=512).

Strategy: data-parallel over batch (128 rows/core x 8 cores). All weights
(~20.5 MB fp32) stay SBUF-resident. Matmuls run as fp32r (fp32 with 11-bit
mantissa inputs, fp32 PSUM accumulation) at 1 column/cycle. Orientation:
activations transposed (feature-major) as the stationary operand, weights as
the moving operand with N=512 per PSUM bank; gate nonlinearities on ScalarE,
elementwise cell updates on VectorE, h re-transposed via TensorE each step.
The 48-step recurrence is fully unrolled (no loop back-edge barriers); x
arrives pre-transposed from the host. The tiny final [B,H]@[H,1] projection
runs on the host.
"""
import sys
import os

for _p in ("/opt/trn_rl_repo", "/root/.axon_site/_ro/trn_rl_repo"):
    if os.path.isdir(_p) and _p not in sys.path:
        sys.path.insert(0, _p)

import numpy as np

import concourse.bass as bass
import concourse.tile as tile
from concourse import bacc, mybir
from concourse import bass_utils
from concourse.bass import ds, ts
from concourse.masks import make_identity

P = 128          # batch rows per core / SBUF partitions
T = 48           # sequence length
IN = 64          # input features
H = 512          # hidden size
L = 3            # layers
G4 = 4 * H       # gate width (2048)
NB = 4           # PSUM banks per gate row (G4 / 512)
KH = H // P      # k-chunks of the hidden contraction (4)
NCORES = 8

F32 = mybir.dt.float32
F32R = mybir.dt.float32r
AF = mybir.ActivationFunctionType


def _round_fp32r(a: np.ndarray) -> np.ndarray:
    """Round fp32 to fp32r (11-bit mantissa) with round-to-nearest-even."""
    u = np.ascontiguousarray(a, dtype=np.float32).view(np.uint32)
    u = u + 0x7FF + ((u >> 12) & 1)
    u &= np.uint32(0xFFFFF000)
    return u.view(np.float32)


def _build(
    include_bias: bool,
    reps: int = 1,
    tp_in_g: bool = True,
    tail_chunk: int = 1,
    act_first: str = "cifo",
    bank_order: tuple = (0, 1, 2, 3),
    c_chunk: int = 2,
) -> bass.Bass:
    """reps>1 wraps the whole recurrence in an outer repeat loop (timing only).

    tp_in_g: write h-transposes into the cell's own g PSUM banks (after the
    gate reads) instead of a separate rotating slot, so the two g slots form a
    true cell-to-cell double buffer.
    tail_chunk: split the elementwise gate tail into this many column chunks
    to pipeline ACT/DVE/PE-transpose.
    """
    nc = bacc.Bacc()

    # x pre-transposed on the host: [128, (T//2)*128] fp32r, even t in
    # partitions 0:64, odd t in 64:128, column block t//2 holds x_t^T.
    xT_d = nc.dram_tensor("xT", [P, (T // 2) * P], F32R, kind="ExternalInput")
    wx_d = [
        nc.dram_tensor("wx0", [IN, G4], F32R, kind="ExternalInput"),
        nc.dram_tensor("wx1", [H, G4], F32R, kind="ExternalInput"),
        nc.dram_tensor("wx2", [H, G4], F32R, kind="ExternalInput"),
    ]
    wh_d = [
        nc.dram_tensor(f"wh{l}", [H, G4], F32R, kind="ExternalInput")
        for l in range(L)
    ]
    b_d = (
        [nc.dram_tensor(f"b{l}", [1, G4], F32R, kind="ExternalInput") for l in range(L)]
        if include_bias
        else None
    )
    # final h of the top layer back to the host (transposed fp32r layout)
    out_d = nc.dram_tensor("hout", [P, H], F32R, kind="ExternalOutput")

    with tile.TileContext(nc) as tc:
        with (
            tc.tile_pool(name="wpool", bufs=1) as wp,
            tc.tile_pool(name="state", bufs=1) as st,
            tc.tile_pool(name="work", bufs=1) as wk,
            tc.tile_pool(name="psg", bufs=2, space="PSUM") as psg,
        ):
            # ---- persistent tiles -------------------------------------------------
            ident = wp.tile([P, P], F32)
            make_identity(nc, ident)

            xT_t = wp.tile([P, (T // 2) * P], F32R)
            nc.sync.dma_start(xT_t[:], xT_d[:])

            # Wx0 duplicated into both partition halves so odd-t x tiles
            # (living at base partition 64) find it on matching partitions.
            wx0_t = wp.tile([P, G4], F32R)
            nc.sync.dma_start(wx0_t[:IN, :], wx_d[0][:])
            nc.sync.dma_start(wx0_t[IN:, :], wx_d[0][:])
            # [H, G4] weights as [128, KH, G4]: partition = k % 128, k-chunk = k // 128
            big_w = {}
            for name, d in (
                ("wh0", wh_d[0]),
                ("wx1", wx_d[1]),
                ("wh1", wh_d[1]),
                ("wx2", wx_d[2]),
                ("wh2", wh_d[2]),
            ):
                w_t = wp.tile([P, KH, G4], F32R, name=f"{name}_t")
                nc.sync.dma_start(w_t[:], d.rearrange("(ko ki) n -> ki ko n", ki=P))
                big_w[name] = w_t

            if include_bias:
                ones_f = wp.tile([1, P], F32)
                nc.vector.memset(ones_f[:], 1.0)
                ones_t = wp.tile([1, P], F32R)
                nc.scalar.copy(ones_t[:], ones_f[:])
                b_t = []
                for l in range(L):
                    bt = wp.tile([1, G4], F32R, name=f"b{l}_t")
                    nc.sync.dma_start(bt[:], b_d[l][:])
                    b_t.append(bt)

            # states: h transposed (feature-major, fp32r), C batch-major (fp32).
            # memset doesn't take fp32r, so zero an fp32 scratch and cast-copy.
            zbuf = wk.tile([P, H], F32, tag="f_s")
            nc.vector.memset(zbuf[:], 0.0)
            hT = []
            Cs = []
            for l in range(L):
                h_t = st.tile([P, H], F32R, name=f"hT{l}")
                nc.scalar.copy(h_t[:], zbuf[:])
                hT.append(h_t)
                c_t = st.tile([P, H], F32, name=f"C{l}")
                nc.vector.memset(c_t[:], 0.0)
                Cs.append(c_t)

            pending_finish = [None]

            def cell(l: int, xin):
                """One LSTM cell update: xin = (lhsT, rhs) pairs of the fresh input.

                The previous cell's transposes + hT copy are emitted between this
                cell's state-half and input-half matmuls, so the PE interleaves
                them without stalling on the previous cell's gate tail.
                """
                # If the pending finish writes this cell's own layer state
                # (only in wavefront startup/teardown ragged slots), its value
                # is this cell's state input: flush before the state matmuls.
                if pending_finish[0] is not None and pending_finish[0][0] == l:
                    pending_finish[0][1]()
                    pending_finish[0] = None
                g = psg.tile([P, G4], F32, name="g", tag="gps")
                state_pairs = [
                    (hT[l][:, ts(j, P)], big_w[f"wh{l}"][:, j]) for j in range(KH)
                ]
                for n in range(NB):
                    if include_bias:
                        nc.tensor.matmul(
                            g[:, ts(n, 512)],
                            ones_t[:],
                            b_t[l][:, ts(n, 512)],
                            start=True,
                            stop=False,
                        )
                    for kidx, (lhsT, rhs) in enumerate(state_pairs):
                        nc.tensor.matmul(
                            g[:, ts(n, 512)],
                            lhsT,
                            rhs[:, ts(n, 512)],
                            start=(kidx == 0 and not include_bias),
                            stop=False,
                            skip_group_check=True,
                        )
                for n in bank_order:
                    nk = len(xin)
                    for kidx, (lhsT, rhs) in enumerate(xin):
                        nc.tensor.matmul(
                            g[:, ts(n, 512)],
                            lhsT,
                            rhs[:, ts(n, 512)],
                            start=False,
                            stop=(kidx == nk - 1),
                            skip_group_check=True,
                        )

                f_s = wk.tile([P, H], F32)
                i_s = wk.tile([P, H], F32)
                c_s = wk.tile([P, H], F32)
                o_s = wk.tile([P, H], F32)
                fC = wk.tile([P, H], F32)
                ic = wk.tile([P, H], F32)
                tanC = wk.tile([P, H], F32, tag="f_s")
                h_b = wk.tile([P, H], F32, tag="i_s")

                gate_tile = {"f": f_s, "i": i_s, "c": c_s, "o": o_s}
                gate_fun = {
                    "f": AF.Sigmoid, "i": AF.Sigmoid, "c": AF.Tanh, "o": AF.Sigmoid,
                }
                gate_bank = {"f": 0, "i": 1, "c": 2, "o": 3}
                W = H // tail_chunk
                for q in range(tail_chunk):
                    s = slice(q * W, (q + 1) * W)
                    for gname in act_first:
                        b = gate_bank[gname]
                        nc.scalar.activation(
                            gate_tile[gname][:, s],
                            g[:, b * H + q * W : b * H + q * W + W],
                            gate_fun[gname],
                        )
                    Wc = W // c_chunk
                    for r in range(c_chunk):
                        sc = slice(q * W + r * Wc, q * W + (r + 1) * Wc)
                        nc.vector.tensor_mul(ic[:, sc], i_s[:, sc], c_s[:, sc])
                        nc.vector.tensor_mul(fC[:, sc], f_s[:, sc], Cs[l][:, sc])
                        nc.vector.tensor_add(Cs[l][:, sc], fC[:, sc], ic[:, sc])
                        nc.scalar.activation(tanC[:, sc], Cs[l][:, sc], AF.Tanh)
                        nc.vector.tensor_mul(h_b[:, sc], o_s[:, sc], tanC[:, sc])

                # h -> transposed fp32r state via PE transpose + cast copy,
                # deferred so it interleaves with the next cell's state matmuls.
                if tp_in_g:
                    # reuse this cell's own g banks (gates already consumed)
                    tp = g
                else:
                    tp = psg.tile([P, H], F32, name="tp", tag="gps")

                def finish(l=l, tp=tp, h_b=h_b):
                    for j in range(KH):
                        nc.tensor.transpose(tp[:, ts(j, P)], h_b[:, ts(j, P)], ident[:])
                    nc.vector.tensor_copy(hT[l][:], tp[:, :H])

                # Flush the previous cell's finish only now, at the end of this
                # cell: every emitted reader of the previous hT version (this
                # cell's state and input matmuls) precedes the new write, which
                # keeps single-buffered hT correct under the wavefront order.
                if pending_finish[0] is not None:
                    pending_finish[0][1]()
                pending_finish[0] = (l, finish)

            def emit_cell(t: int, l: int):
                if l == 0:
                    r0 = 0 if t % 2 == 0 else 64
                    xin = [(xT_t[r0 : r0 + IN, ts(t // 2, P)], wx0_t[r0 : r0 + IN, :])]
                else:
                    xin = [
                        (hT[l - 1][:, ts(j, P)], big_w[f"wx{l}"][:, j])
                        for j in range(KH)
                    ]
                cell(l, xin)

            def whole_pass():
                # wavefront order: cells (s,0), (s-1,1), (s-2,2) — every
                # cell's consumers are >= 3 cells later, so gate tails hide
                # under other cells' matmuls.
                for s in range(T + L - 1):
                    for l in range(L):
                        t = s - l
                        if 0 <= t < T:
                            emit_cell(t, l)
                if pending_finish[0] is not None:
                    pending_finish[0][1]()
                    pending_finish[0] = None

            if reps > 1:
                with tc.For_i(0, reps, 1):
                    whole_pass()
            else:
                whole_pass()

            if pending_finish[0] is not None:
                pending_finish[0]()
                pending_finish[0] = None

            # ---- ship final top-layer h back to the host --------------------------
            nc.sync.dma_start(out_d[:], hT[L - 1][:])

    nc.finalize()
    return nc


_NC_CACHE: dict = {}
_LAST_RUN: dict = {}


def _pack_xT(x_shard: np.ndarray) -> np.ndarray:
    """[128, T, IN] -> [128, (T//2)*128] packed transposed layout (fp32r-rounded)."""
    xt = np.zeros((P, (T // 2) * P), dtype=np.float32)
    for t in range(T):
        r0 = 0 if t % 2 == 0 else 64
        xt[r0 : r0 + IN, (t // 2) * P : (t // 2 + 1) * P] = x_shard[:, t, :].T
    return _round_fp32r(xt)


def kernel(**inputs) -> np.ndarray:
    x = np.ascontiguousarray(np.asarray(inputs["x"], dtype=np.float32))
    B = x.shape[0]
    assert B % NCORES == 0
    Bl = B // NCORES

    ws = {}
    for name in ("Wx0", "Wh0", "Wx1", "Wh1", "Wx2", "Wh2"):
        ws[name] = _round_fp32r(np.asarray(inputs[name], dtype=np.float32))
    fc_w = np.asarray(inputs["fc_w"], dtype=np.float32)
    bs = [np.asarray(inputs[f"b{l}"], dtype=np.float32) for l in range(L)]
    fc_b = np.asarray(inputs["fc_b"], dtype=np.float32)
    include_bias = any(np.any(b != 0) for b in bs)

    key = include_bias
    if key not in _NC_CACHE:
        _NC_CACHE[key] = _build(include_bias)
    nc = _NC_CACHE[key]
    _LAST_RUN["include_bias"] = include_bias

    in_maps = []
    for c in range(NCORES):
        m = {
            "xT": _pack_xT(x[c * Bl : (c + 1) * Bl]),
            "wx0": ws["Wx0"],
            "wx1": ws["Wx1"],
            "wx2": ws["Wx2"],
            "wh0": ws["Wh0"],
            "wh1": ws["Wh1"],
            "wh2": ws["Wh2"],
        }
        if include_bias:
            for l in range(L):
                m[f"b{l}"] = _round_fp32r(bs[l]).reshape(1, G4)
        in_maps.append(m)

    res = bass_utils.run_bass_kernel_spmd(nc, in_maps, core_ids=list(range(NCORES)))
    _LAST_RUN["nc"] = nc
    _LAST_RUN["in_maps"] = in_maps
    outs = []
    for c in range(NCORES):
        ht = res.results[c]["hout"]  # [128, 512]: ht[p, 128*j + b] = h2[b, 128*j + p]
        h2 = ht.reshape(P, KH, P).transpose(2, 1, 0).reshape(P, H)
        outs.append(h2 @ fc_w)
    out = np.concatenate(outs, axis=0)
    return (out + fc_b.reshape(1, -1)).astype(np.float32)
